# revision 15
# baseline (speedup 1.0000x reference)
"""Trainium2 Bass kernel for nn_MentionPruner (topk_masking).

Data-parallel over batch: 8 documents -> 8 NeuronCores. Per core:
  - 3-layer MLP scorer (fp32 matmuls on PE, X transposed on-chip via PE)
  - mask penalty, exact top-L selection via threshold bisection
  - compaction: per-partition ascending extraction (max/max_index/match_replace
    8-at-a-time) + telescoped prefix matmuls + row-granular indirect-DMA gather
  - indirect-DMA gathers for pruned vecs/scores/begin/end
  - outer-product validity masks + tril, spans from idx arithmetic

Self-contained: hardcodes all shapes; host only shards inputs, computes
span_lengths (= floor(seqlen*0.2)+1, mirrored fp32 math) and stacks outputs.
"""
import os
import sys

sys.path.insert(0, "/opt/trn_rl_repo")

import numpy as np
import concourse.bass as bass
import concourse.mybir as mybir
import concourse.tile as tile
from concourse import bacc
from concourse.bass_utils import run_bass_kernel_spmd
from concourse.masks import make_identity

# ---- problem constants ----
B = 8
SEQ = 2048
MSL = 5
N = SEQ * MSL          # 10240 spans
D = 2048
H = 512
K = 410                # max kept spans
P = 128
NCOL = N // P          # 80 score columns per partition
G = 512                # span-group (moving dim) for MLP
NG = N // G            # 20 groups
KT1 = D // P           # 16
MT = H // P            # 4
KT2 = H // P           # 4
NITER = 38             # bisection iterations: 20010 * 2^-38 ~ 7e-8 << min gap
MAGIC = 8388608.0      # 2^23, for exact fp32 floor
BIG = 1.0e6
KROW = 512             # padded output-slot row (>= K, multiple of 128)

f32 = mybir.dt.float32
i32 = mybir.dt.int32
u32 = mybir.dt.uint32
Alu = mybir.AluOpType
Act = mybir.ActivationFunctionType


def _floor(nc, pool, out, in_, tag):
    """out = floor(in_) for 0 <= in_ < 2^22. Exact fp32 round-to-nearest trick."""
    sh = list(in_.shape)
    r = pool.tile(sh, f32, tag=f"{tag}_r")
    g = pool.tile(sh, f32, tag=f"{tag}_g")
    nc.vector.tensor_scalar(r[:], in_, MAGIC, None, op0=Alu.add)
    nc.vector.tensor_scalar(r[:], r[:], MAGIC, None, op0=Alu.subtract)
    nc.vector.tensor_tensor(g[:], r[:], in_, op=Alu.is_gt)
    nc.vector.tensor_tensor(out, r[:], g[:], op=Alu.subtract)


def emit_tail(nc, tc, sbc, tail, smallps, s_pm, lf_sb, Lb, x_in, sbegin_in, send_in, outs):
    """Everything after the penalized scores s_pm [P, NCOL] are in SBUF.

    sbc: const pool, tail: work pool, smallps: PSUM pool (tag 'small').
    outs: dict of output DRAM handles.
    """
    # --- constants ---
    ones_sq = sbc.tile([P, P], f32)  # all-ones: matmul -> total replicated to all partitions
    nc.vector.memset(ones_sq[:], 1.0)
    triu = sbc.tile([P, P], f32)  # triu[p, j] = 1 iff p < j (strict upper)
    nc.vector.memset(triu[:], 1.0)
    nc.gpsimd.affine_select(
        out=triu[:], in_=triu[:], pattern=[[1, P]], compare_op=Alu.is_gt,
        fill=0.0, base=0, channel_multiplier=-1)  # keep where c - p > 0
    # superdiagonal shift matrix: S[p, j] = 1 iff j == p + 1
    shiftm = sbc.tile([P, P], f32)
    nc.vector.memset(shiftm[:], 1.0)
    nc.gpsimd.affine_select(  # keep c >= p + 1
        out=shiftm[:], in_=shiftm[:], pattern=[[1, P]], compare_op=Alu.is_ge,
        fill=0.0, base=-1, channel_multiplier=-1)
    nc.gpsimd.affine_select(  # keep c <= p + 1
        out=shiftm[:], in_=shiftm[:], pattern=[[-1, P]], compare_op=Alu.is_ge,
        fill=0.0, base=1, channel_multiplier=1)
    # e80[p] = NCOL for p >= 1 else 0   (telescopes to NCOL * p_k)
    e80 = sbc.tile([P, 1], f32)
    nc.vector.memset(e80[:], float(NCOL))
    nc.gpsimd.affine_select(  # keep where p >= 1
        out=e80[:], in_=e80[:], pattern=[[0, 1]], compare_op=Alu.is_ge,
        fill=0.0, base=-1, channel_multiplier=1)
    # k iota broadcast to all partitions [P, KROW], and slot grid k = g*128 + p
    kiota_b = sbc.tile([P, KROW], i32)
    nc.gpsimd.iota(kiota_b[:], pattern=[[1, KROW]], base=0, channel_multiplier=0)
    kiota_bf = sbc.tile([P, KROW], f32)
    nc.vector.tensor_copy(kiota_bf[:], kiota_b[:])
    kgrid = sbc.tile([P, 4], i32)  # k = p + 128*g
    nc.gpsimd.iota(kgrid[:], pattern=[[P, 4]], base=0, channel_multiplier=1)
    kgridf = sbc.tile([P, 4], f32)
    nc.vector.tensor_copy(kgridf[:], kgrid[:])
    pbase = sbc.tile([P, 1], f32)  # p * NCOL
    nc.gpsimd.iota(pbase[:], pattern=[[0, 1]], base=0, channel_multiplier=NCOL,
                   allow_small_or_imprecise_dtypes=True)
    trilg = sbc.tile([P, 4, K], f32)  # per row-group tril masks
    nc.vector.memset(trilg[:], 1.0)
    for g in range(4):
        nc.gpsimd.affine_select(
            out=trilg[:, g, :], in_=trilg[:, g, :], pattern=[[-1, K]],
            compare_op=Alu.is_ge, fill=0.0, base=P * g, channel_multiplier=1)
    # valid_row[0, k] = (k < L)
    valid_row = sbc.tile([1, KROW], f32)
    nc.vector.tensor_scalar(valid_row[:], kiota_bf[:1, :], lf_sb[:, :], None, op0=Alu.is_lt)

    # --- bisection for threshold lo in (v_{L+1}, v_L] ---
    lo = tail.tile([P, 1], f32, tag="lo")
    hi = tail.tile([P, 1], f32, tag="hi")
    nc.vector.memset(lo[:], -10005.0)
    nc.vector.memset(hi[:], 10005.0)
    for _ in range(NITER):
        mid = tail.tile([P, 1], f32, tag="mid")
        nc.vector.tensor_tensor(mid[:], lo[:], hi[:], op=Alu.add)
        nc.vector.tensor_scalar_mul(mid[:], mid[:], 0.5)
        ge = tail.tile([P, NCOL], f32, tag="ge")
        nc.vector.tensor_scalar(ge[:], s_pm[:], mid[:, :], None, op0=Alu.is_ge)
        rowcnt = tail.tile([P, 1], f32, tag="rowcnt")
        nc.vector.tensor_reduce(rowcnt[:], ge[:], axis=mybir.AxisListType.X, op=Alu.add)
        cps = smallps.tile([P, 1], f32, tag="small")
        nc.tensor.matmul(cps[:], lhsT=ones_sq[:], rhs=rowcnt[:], start=True, stop=True)
        cond = tail.tile([P, 1], i32, tag="cond")
        nc.vector.tensor_tensor(cond[:], cps[:], Lb[:], op=Alu.is_ge)
        ncond = tail.tile([P, 1], i32, tag="ncond")
        nc.vector.tensor_tensor(ncond[:], cps[:], Lb[:], op=Alu.is_lt)
        nc.vector.copy_predicated(lo[:], cond[:], mid[:])
        nc.vector.copy_predicated(hi[:], ncond[:], mid[:])

    # --- keep mask + per-partition counts/prefixes ---
    keep = tail.tile([P, NCOL], f32, tag="keep")
    nc.vector.tensor_scalar(keep[:], s_pm[:], lo[:, :], None, op0=Alu.is_ge)
    rowtot = tail.tile([P, 1], f32, tag="rowtot")
    nc.vector.tensor_reduce(rowtot[:], keep[:], axis=mybir.AxisListType.X, op=Alu.add)
    eps = smallps.tile([P, 1], f32, tag="small")
    nc.tensor.matmul(eps[:], lhsT=triu[:], rhs=rowtot[:], start=True, stop=True)
    rowexcl = tail.tile([P, 1], f32, tag="rowexcl")
    nc.scalar.copy(rowexcl[:], eps[:])
    dps = smallps.tile([P, 1], f32, tag="small")
    nc.tensor.matmul(dps[:], lhsT=shiftm[:], rhs=rowtot[:], start=True, stop=True)
    drow = tail.tile([P, 1], f32, tag="drow")
    nc.scalar.copy(drow[:], dps[:])

    # --- per-partition ascending extraction of kept column positions ---
    # neg_z[p, c] = -c if kept else -BIG; 8 smallest-c at a time via max8
    negc = tail.tile([P, NCOL], f32, tag="negc")
    nc.gpsimd.iota(negc[:], pattern=[[-1, NCOL]], base=0, channel_multiplier=0,
                   allow_small_or_imprecise_dtypes=True)  # -c exact in f32
    neg_z = tail.tile([P, NCOL], f32, tag="negz")
    nc.vector.tensor_scalar(neg_z[:], keep[:], BIG, -BIG, op0=Alu.mult, op1=Alu.add)
    nc.vector.tensor_tensor(neg_z[:], neg_z[:], negc[:], op=Alu.add)
    # kept: -c ; not kept: -BIG - c  (all distinct, kept always larger)
    wq = tail.tile([P, NCOL], u32, tag="wq")
    for r in range(NCOL // 8):
        v8 = tail.tile([P, 8], f32, tag="v8")
        nc.vector.max(out=v8[:], in_=neg_z[:])
        nc.vector.max_index(wq[:, r * 8 : (r + 1) * 8], v8[:], neg_z[:])
        if r < NCOL // 8 - 1:
            nc.vector.match_replace(
                out=neg_z[:], in_to_replace=v8[:], in_values=neg_z[:],
                imm_value=-2.0 * BIG)
    # W flat index = p*NCOL + c, as int32, to DRAM for the slot gather
    wf = tail.tile([P, NCOL], f32, tag="wf")
    nc.vector.tensor_copy(wf[:], wq[:])
    nc.vector.tensor_scalar(wf[:], wf[:], pbase[:, :], None, op0=Alu.add)
    wi = tail.tile([P, NCOL], i32, tag="wi")
    nc.vector.tensor_copy(wi[:], wf[:])

    # --- output slot -> source partition/rank, via telescoped matmuls ---
    # cmp[p, k] = (k >= rowexcl[p]);  rowexcl non-decreasing =>
    #   sum_p cmp*drow = rowexcl[p_k],  sum_p cmp*e80 = NCOL*p_k
    cmp = tail.tile([P, KROW], f32, tag="cmp")
    nc.vector.tensor_scalar(cmp[:], kiota_bf[:], rowexcl[:, :], None, op0=Alu.is_ge)
    sel_ps = smallps.tile([1, KROW], f32, tag="small")
    nc.tensor.matmul(sel_ps[:], lhsT=drow[:], rhs=cmp[:], start=True, stop=True)
    rex_row = tail.tile([1, KROW], f32, tag="rexrow")
    nc.scalar.copy(rex_row[:], sel_ps[:])
    sel_ps2 = smallps.tile([1, KROW], f32, tag="small")
    nc.tensor.matmul(sel_ps2[:], lhsT=e80[:], rhs=cmp[:], start=True, stop=True)
    o_row = tail.tile([1, KROW], f32, tag="orow")
    nc.scalar.copy(o_row[:], sel_ps2[:])
    # o = NCOL*p_k + (k - rowexcl[p_k]), clamped to N-1
    nc.vector.tensor_tensor(o_row[:], o_row[:], kiota_bf[:1, :], op=Alu.add)
    nc.vector.tensor_tensor(o_row[:], o_row[:], rex_row[:], op=Alu.subtract)
    nc.vector.tensor_scalar(o_row[:], o_row[:], float(N - 1), None, op0=Alu.min)
    o_rowi = tail.tile([1, KROW], i32, tag="orowi")
    nc.vector.tensor_copy(o_rowi[:], o_row[:])

    with tc.tile_pool(name="tail_dram", bufs=1, space="DRAM") as tdram:
        w_dram = tdram.tile([N, 1], i32)
        nc.sync.dma_start(
            w_dram[:, 0].rearrange("(p f) -> p f", p=P, f=NCOL), wi[:])
        # redistribute o to slot grid o_g[p, g] = o[g*128 + p], via DRAM
        # (SBUF free-dim cannot be reinterpreted as partitions)
        o_dram = tdram.tile([KROW, 1], i32)
        nc.sync.dma_start(o_dram[:, 0].rearrange("(one f) -> one f", one=1, f=KROW), o_rowi[:])
        o_g = tail.tile([P, 4], i32, tag="og")
        nc.sync.dma_start(
            o_g[:], o_dram[:, 0].rearrange("(f p) -> p f", p=P, f=4))

        # gather idx values then override invalid slots with N-1
        idxg = tail.tile([P, 4], i32, tag="idxg")
        for g in range(4):
            nc.gpsimd.indirect_dma_start(
                out=idxg[:, g : g + 1], out_offset=None, in_=w_dram[:, :],
                in_offset=bass.IndirectOffsetOnAxis(ap=o_g[:, g : g + 1], axis=0))
        invg = tail.tile([P, 4], i32, tag="invg")
        nc.vector.tensor_scalar(invg[:], kgridf[:], Lb[:, :], None, op0=Alu.is_ge)
        n1t = tail.tile([P, 4], i32, tag="n1t")
        nc.vector.memset(n1t[:], N - 1)
        nc.vector.copy_predicated(idxg[:], invg[:], n1t[:])

        # idx output [K]
        nc.sync.dma_start(
            outs["idx"][0:384].rearrange("(f p) -> p f", p=P, f=3), idxg[:, 0:3])
        nc.sync.dma_start(
            outs["idx"][384:410].rearrange("(one p) -> p one", one=1, p=26),
            idxg[0:26, 3:4])

        # spans: q = idx // 5, w = idx % 5 -> [begin, begin + w]
        idxf = tail.tile([P, 4], f32, tag="idxf")
        nc.vector.tensor_copy(idxf[:], idxg[:])
        q0 = tail.tile([P, 4], f32, tag="q0")
        nc.vector.tensor_scalar_mul(q0[:], idxf[:], 0.2)
        fq = tail.tile([P, 4], f32, tag="fq")
        _floor(nc, tail, fq[:], q0[:], "fl1")
        t5 = tail.tile([P, 4], f32, tag="t5")
        nc.vector.tensor_scalar_mul(t5[:], fq[:], 5.0)
        wid = tail.tile([P, 4], f32, tag="wid")
        nc.vector.tensor_tensor(wid[:], idxf[:], t5[:], op=Alu.subtract)
        et = tail.tile([P, 4], f32, tag="et")
        nc.vector.tensor_tensor(et[:], fq[:], wid[:], op=Alu.add)
        bi = tail.tile([P, 4], i32, tag="bi")
        nc.vector.tensor_copy(bi[:], fq[:])
        ei = tail.tile([P, 4], i32, tag="ei")
        nc.vector.tensor_copy(ei[:], et[:])
        for col, src in ((0, bi), (1, ei)):
            nc.sync.dma_start(
                outs["spans"][0:384, col].rearrange("(f p) -> p f", p=P, f=3),
                src[:, 0:3])
            nc.sync.dma_start(
                outs["spans"][384:410, col].rearrange("(one p) -> p one", one=1, p=26),
                src[0:26, 3:4])

        # gathers: pruned_vecs rows (8KB each) + elementwise scores/begin/end
        with tc.tile_pool(name="gpool", bufs=2) as gpool:
            for g in range(4):
                rows = P if g < 3 else K - 3 * P
                vec_sb = gpool.tile([P, D], f32, tag="vec")
                nc.gpsimd.indirect_dma_start(
                    out=vec_sb[:],
                    out_offset=None,
                    in_=x_in[:, :],
                    in_offset=bass.IndirectOffsetOnAxis(ap=idxg[:, g : g + 1], axis=0),
                )
                nc.sync.dma_start(
                    outs["pruned_vecs"][g * P : g * P + rows, :], vec_sb[0:rows, :])

        psc = tail.tile([P, 4], f32, tag="psc")
        pbg = tail.tile([P, 4], i32, tag="pbg")
        peg = tail.tile([P, 4], i32, tag="peg")
        scores_pm_dram = tdram.tile([N, 1], f32)
        nc.sync.dma_start(
            scores_pm_dram[:, 0].rearrange("(p f) -> p f", p=P, f=NCOL), s_pm[:])
        for g in range(4):
            nc.gpsimd.indirect_dma_start(
                out=psc[:, g : g + 1], out_offset=None, in_=scores_pm_dram[:, :],
                in_offset=bass.IndirectOffsetOnAxis(ap=idxg[:, g : g + 1], axis=0))
            nc.gpsimd.indirect_dma_start(
                out=pbg[:, g : g + 1], out_offset=None, in_=sbegin_in[:, :],
                in_offset=bass.IndirectOffsetOnAxis(ap=idxg[:, g : g + 1], axis=0))
            nc.gpsimd.indirect_dma_start(
                out=peg[:, g : g + 1], out_offset=None, in_=send_in[:, :],
                in_offset=bass.IndirectOffsetOnAxis(ap=idxg[:, g : g + 1], axis=0))
        for name, src in (("pruned_scores", psc), ("pruned_begin", pbg), ("pruned_end", peg)):
            nc.sync.dma_start(
                outs[name][0:384, 0].rearrange("(f p) -> p f", p=P, f=3), src[:, 0:3])
            nc.sync.dma_start(
                outs[name][384:410, 0].rearrange("(one p) -> p one", one=1, p=26),
                src[0:26, 3:4])

        # masks: square = valid x valid^T (outer product), tri = square * tril
        for g in range(4):
            rows = P if g < 3 else K - 3 * P
            mp = smallps.tile([P, K], f32, tag="small")
            nc.tensor.matmul(
                mp[:], lhsT=valid_row[:, g * P : (g + 1) * P],
                rhs=valid_row[:, 0:K], start=True, stop=True)
            sq_sb = tail.tile([P, K], f32, tag="sq")
            nc.vector.tensor_copy(sq_sb[:], mp[:])
            tri_sb = tail.tile([P, K], f32, tag="tri")
            nc.vector.tensor_tensor(tri_sb[:], sq_sb[:], trilg[:, g, :], op=Alu.mult)
            nc.sync.dma_start(outs["square_mask"][g * P : g * P + rows, :], sq_sb[0:rows, :])
            nc.sync.dma_start(outs["triangular_mask"][g * P : g * P + rows, :], tri_sb[0:rows, :])


def declare_io(nc, tail_only=False):
    ins = {}
    outs = {}
    if tail_only:
        ins["scores_raw"] = nc.declare_dram_parameter("scores_raw", [N, 1], f32, isOutput=False)
    else:
        ins["W1"] = nc.declare_dram_parameter("W1", [D, H], f32, isOutput=False)
        ins["b1"] = nc.declare_dram_parameter("b1", [H], f32, isOutput=False)
        ins["W2"] = nc.declare_dram_parameter("W2", [H, H], f32, isOutput=False)
        ins["b2"] = nc.declare_dram_parameter("b2", [H], f32, isOutput=False)
        ins["W3"] = nc.declare_dram_parameter("W3", [H, 1], f32, isOutput=False)
        ins["b3"] = nc.declare_dram_parameter("b3", [1], f32, isOutput=False)
    ins["x"] = nc.declare_dram_parameter("x", [N, D], f32, isOutput=False)
    ins["mask"] = nc.declare_dram_parameter("mask", [N], f32, isOutput=False)
    ins["sbegin"] = nc.declare_dram_parameter("sbegin", [N, 1], i32, isOutput=False)
    ins["send"] = nc.declare_dram_parameter("send", [N, 1], i32, isOutput=False)
    ins["lf"] = nc.declare_dram_parameter("lf", [1], f32, isOutput=False)

    outs["prune_scores"] = nc.declare_dram_parameter("prune_scores", [N, 1], f32, isOutput=True)
    outs["idx"] = nc.declare_dram_parameter("idx", [K], i32, isOutput=True)
    outs["pruned_vecs"] = nc.declare_dram_parameter("pruned_vecs", [K, D], f32, isOutput=True)
    outs["pruned_scores"] = nc.declare_dram_parameter("pruned_scores", [K, 1], f32, isOutput=True)
    outs["pruned_begin"] = nc.declare_dram_parameter("pruned_begin", [K, 1], i32, isOutput=True)
    outs["pruned_end"] = nc.declare_dram_parameter("pruned_end", [K, 1], i32, isOutput=True)
    outs["square_mask"] = nc.declare_dram_parameter("square_mask", [K, K], f32, isOutput=True)
    outs["triangular_mask"] = nc.declare_dram_parameter("triangular_mask", [K, K], f32, isOutput=True)
    outs["spans"] = nc.declare_dram_parameter("spans", [K, 2], i32, isOutput=True)
    return ins, outs


def _penalize_and_tail(nc, tc, sbc, tail, smallps, s_raw, ins, outs):
    """s_raw [P, NCOL] raw scores -> penalty -> prune_scores out -> tail."""
    lf_sb = sbc.tile([1, 1], f32)
    nc.sync.dma_start(lf_sb[:], ins["lf"][:].rearrange("(p one) -> p one", p=1, one=1))
    Lb = sbc.tile([P, 1], f32)
    nc.gpsimd.partition_broadcast(Lb[:], lf_sb[:])

    mask_t = tail.tile([P, NCOL], f32, tag="mask")
    nc.sync.dma_start(mask_t[:], ins["mask"][:].rearrange("(p f) -> p f", p=P, f=NCOL))
    pen = tail.tile([P, NCOL], f32, tag="pen")
    nc.vector.tensor_scalar(pen[:], mask_t[:], 10000.0, -10000.0, op0=Alu.mult, op1=Alu.add)
    s_pm = tail.tile([P, NCOL], f32, tag="spm")
    nc.vector.tensor_tensor(s_pm[:], s_raw[:], pen[:], op=Alu.add)
    nc.sync.dma_start(
        outs["prune_scores"][:, 0].rearrange("(p f) -> p f", p=P, f=NCOL), s_pm[:])
    emit_tail(nc, tc, sbc, tail, smallps, s_pm, lf_sb, Lb,
              ins["x"], ins["sbegin"], ins["send"], outs)


def build_full():
    ng = int(os.environ.get("KB_NG") or NG)
    skip_tail = bool(os.environ.get("KB_SKIP_TAIL"))
    skip_tp = bool(os.environ.get("KB_SKIP_TP"))
    nc = bacc.Bacc(None)
    ins, outs = declare_io(nc, tail_only=False)
    with tile.TileContext(nc) as tc:
        with (
            tc.tile_pool(name="const", bufs=1) as sbc,
            tc.tile_pool(name="xg", bufs=3) as xgp,
            tc.tile_pool(name="xt", bufs=1) as xtp,
            tc.tile_pool(name="h1", bufs=2) as h1p,
            tc.tile_pool(name="h2", bufs=2) as h2p,
            tc.tile_pool(name="scr", bufs=2) as scrp,
            tc.tile_pool(name="tail", bufs=2) as tail,
            tc.tile_pool(name="tp_ps", bufs=2, space="PSUM") as tpps,
            tc.tile_pool(name="ps1", bufs=2, space="PSUM") as ps1p,
            tc.tile_pool(name="ps2", bufs=2, space="PSUM") as ps2p,
            tc.tile_pool(name="smallps", bufs=2, space="PSUM") as smallps,
            tc.tile_pool(name="mlp_dram", bufs=1, space="DRAM") as mdram,
        ):
            ident = sbc.tile([P, P], f32)
            make_identity(nc, ident[:])

            # weights, layouts: w[p, k, m*128+j] = W[k*128+p, m*128+j]
            w1_sb = sbc.tile([P, KT1, H], f32)
            nc.sync.dma_start(
                w1_sb[:], ins["W1"][:].rearrange("(k p) h -> p k h", p=P, k=KT1))
            w2_sb = sbc.tile([P, KT2, H], f32)
            nc.sync.dma_start(
                w2_sb[:], ins["W2"][:].rearrange("(k p) h -> p k h", p=P, k=KT2))
            w3_sb = sbc.tile([P, KT2], f32)
            nc.sync.dma_start(
                w3_sb[:], ins["W3"][:, 0].rearrange("(k p) -> p k", p=P, k=KT2))
            b1_sb = sbc.tile([P, MT], f32)
            nc.sync.dma_start(b1_sb[:], ins["b1"][:].rearrange("(m p) -> p m", p=P, m=MT))
            b2_sb = sbc.tile([P, MT], f32)
            nc.sync.dma_start(b2_sb[:], ins["b2"][:].rearrange("(m p) -> p m", p=P, m=MT))
            b3_sb = sbc.tile([1, 1], f32)
            nc.sync.dma_start(b3_sb[:], ins["b3"][:].rearrange("(p one) -> p one", p=1, one=1))

            scores_raw_dram = mdram.tile([1, N], f32)

            for g in range(ng):
                xt_t = xtp.tile([P, KT1, G], f32, tag="xt")
                for s in range(G // P):
                    xg_t = xgp.tile([P, D], f32, tag="xg")
                    nc.sync.dma_start(
                        xg_t[:], ins["x"][g * G + s * P : g * G + (s + 1) * P, :])
                    for k in range(KT1):
                        if skip_tp:
                            nc.vector.tensor_copy(
                                xt_t[:, k, s * P : (s + 1) * P],
                                xg_t[:, k * P : (k + 1) * P])
                            continue
                        pst = tpps.tile([P, P], f32, tag="tp")
                        nc.tensor.transpose(pst[:], xg_t[:, k * P : (k + 1) * P], ident[:])
                        nc.vector.tensor_copy(xt_t[:, k, s * P : (s + 1) * P], pst[:])
                h1_t = h1p.tile([P, MT, G], f32, tag="h1")
                for m in range(MT):
                    ps = ps1p.tile([P, G], f32, tag="ps1")
                    for k in range(KT1):
                        nc.tensor.matmul(
                            ps[:], lhsT=w1_sb[:, k, m * P : (m + 1) * P],
                            rhs=xt_t[:, k, :], start=(k == 0), stop=(k == KT1 - 1))
                    nc.scalar.activation(h1_t[:, m, :], ps[:], Act.Relu, bias=b1_sb[:, m : m + 1])
                h2_t = h2p.tile([P, MT, G], f32, tag="h2")
                for m in range(MT):
                    ps = ps2p.tile([P, G], f32, tag="ps2")
                    for k in range(KT2):
                        nc.tensor.matmul(
                            ps[:], lhsT=w2_sb[:, k, m * P : (m + 1) * P],
                            rhs=h1_t[:, k, :], start=(k == 0), stop=(k == KT2 - 1))
                    nc.scalar.activation(h2_t[:, m, :], ps[:], Act.Relu, bias=b2_sb[:, m : m + 1])
                ps3 = smallps.tile([1, G], f32, tag="small")
                for k in range(KT2):
                    nc.tensor.matmul(
                        ps3[:], lhsT=w3_sb[:, k : k + 1], rhs=h2_t[:, k, :],
                        start=(k == 0), stop=(k == KT2 - 1))
                sc_t = scrp.tile([1, G], f32, tag="sc")
                nc.vector.tensor_scalar(sc_t[:], ps3[:], b3_sb[:, :], None, op0=Alu.add)
                nc.sync.dma_start(scores_raw_dram[:, g * G : (g + 1) * G], sc_t[:])

            if not skip_tail:
                s_raw = tail.tile([P, NCOL], f32, tag="sraw")
                nc.sync.dma_start(
                    s_raw[:], scores_raw_dram[0, :].rearrange("(p f) -> p f", p=P, f=NCOL))
                _penalize_and_tail(nc, tc, sbc, tail, smallps, s_raw, ins, outs)
    nc.finalize()
    return nc


def build_tail_only():
    nc = bacc.Bacc(None)
    ins, outs = declare_io(nc, tail_only=True)
    with tile.TileContext(nc) as tc:
        with (
            tc.tile_pool(name="const", bufs=1) as sbc,
            tc.tile_pool(name="tail", bufs=2) as tail,
            tc.tile_pool(name="smallps", bufs=2, space="PSUM") as smallps,
        ):
            s_raw = tail.tile([P, NCOL], f32, tag="sraw")
            nc.sync.dma_start(
                s_raw[:], ins["scores_raw"][:, 0].rearrange("(p f) -> p f", p=P, f=NCOL))
            _penalize_and_tail(nc, tc, sbc, tail, smallps, s_raw, ins, outs)
    nc.finalize()
    return nc


_NC_CACHE = {}


def _get_nc(kind):
    if kind not in _NC_CACHE:
        _NC_CACHE[kind] = build_full() if kind == "full" else build_tail_only()
    return _NC_CACHE[kind]


OUT_NAMES = ["prune_scores", "idx", "pruned_vecs", "pruned_scores", "pruned_begin",
             "pruned_end", "square_mask", "triangular_mask", "spans"]


def kernel(span_vecs, span_mask, span_begin, span_end, sequence_lengths,
           W1, b1, W2, b2, W3, b3):
    span_vecs = np.ascontiguousarray(np.asarray(span_vecs, dtype=np.float32))
    span_mask = np.ascontiguousarray(np.asarray(span_mask, dtype=np.float32))
    span_begin = np.ascontiguousarray(np.asarray(span_begin, dtype=np.int32))
    span_end = np.ascontiguousarray(np.asarray(span_end, dtype=np.int32))
    seq = np.asarray(sequence_lengths)
    span_lengths = (seq.astype(np.float32) * np.float32(0.2)).astype(np.int32) + 1
    W1 = np.ascontiguousarray(np.asarray(W1, dtype=np.float32))
    b1 = np.ascontiguousarray(np.asarray(b1, dtype=np.float32))
    W2 = np.ascontiguousarray(np.asarray(W2, dtype=np.float32))
    b2 = np.ascontiguousarray(np.asarray(b2, dtype=np.float32))
    W3 = np.ascontiguousarray(np.asarray(W3, dtype=np.float32))
    b3 = np.ascontiguousarray(np.asarray(b3, dtype=np.float32))

    in_maps = []
    for b in range(B):
        in_maps.append({
            "x": span_vecs[b],
            "mask": span_mask[b],
            "sbegin": span_begin[b].reshape(N, 1),
            "send": span_end[b].reshape(N, 1),
            "lf": np.array([span_lengths[b]], np.float32),
            "W1": W1, "b1": b1, "W2": W2, "b2": b2, "W3": W3, "b3": b3,
        })

    nc = _get_nc("full")
    trace = bool(os.environ.get("KERNEL_TRACE"))
    res = run_bass_kernel_spmd(nc, in_maps, core_ids=list(range(B)), trace=trace)
    if trace and res.exec_time_ns is not None:
        print(f"HW exec time: {res.exec_time_ns} ns", flush=True)
        kernel.last_exec_time_ns = res.exec_time_ns
        kernel.last_results = res

    rs = res.results
    stacked = {name: np.stack([rs[b][name] for b in range(B)]) for name in OUT_NAMES}
    return (
        stacked["prune_scores"],
        stacked["idx"],
        stacked["pruned_vecs"],
        stacked["pruned_scores"],
        stacked["pruned_begin"],
        stacked["pruned_end"],
        span_lengths.astype(np.int32),
        stacked["square_mask"],
        stacked["triangular_mask"],
        stacked["spans"],
    )
